# revision 30
# baseline (speedup 1.0000x reference)
"""Trainium2 Bass kernel for the Aligner2 problem.

Computes, for each batch b:
  k = LReLU(conv3(LReLU(conv3(keys))))        # [256, 520] (pad 3, kernel 3 twice)
  q = LReLU(conv7(LReLU(conv7(LReLU(conv7(queries))))))  # [256, 2048]
  raw[t,s]  = sum_c q[c,t] k[c,s] - 0.5*k2[s]
  l = 2*TEMP*raw   (the -TEMP*q2 term cancels in log_softmax)
  logp = l - logsumexp_s(l);  attn = exp(logp)

v2: query convs 2+3 run in fp8e4 with DoubleRow perf mode (2 k-tiles per
matmul, 0.5 cyc/row); the -0.5*k2 row is broadcast-added on DVE instead of
per-tile PE matmuls; logp computed as SC*raw - ln(z) on DVE (no big Ln on
scalar). Everything else (key path, qconv1, qk scores) stays bf16.

Sharded data-parallel over batch across 8 NeuronCores (4 batches/core).
"""
import numpy as np

import concourse.bass as bass
import concourse.bacc as bacc
import concourse.tile as tile
from concourse import mybir
from concourse.bass_utils import run_bass_kernel_spmd

F32 = mybir.dt.float32
BF16 = mybir.dt.bfloat16
F8 = mybir.dt.float8e4
AF = mybir.ActivationFunctionType
DR = mybir.MatmulPerfMode.DoubleRow

SLOPE = 0.3
TEMPERATURE = 0.0005
SC = 2.0 * TEMPERATURE  # scale applied to the raw PE scores
SW = 256.0              # fp8 weight pre-scale (host side); act descales

BPC = 4          # batches per core
N_CORES = 8
D_DEC, TQ = 80, 2048
D_ENC, TK = 512, 512
DH = 256
TK1 = TK + 4     # 516 after key conv1 (kernel 3, pad 3)
TK2 = TK + 8     # 520 after key conv2
HT1 = TK1 // 2   # 258
HT2 = TK2 // 2   # 260
TQP = TQ + 16    # fp8 padded q activations: stride multiple of 16

# act_info.json set containing Prelu, Exp, Ln, Copy, Identity together
ACT_SET_ALL = 6  # natural_log_exp_and_others

DT_MM = BF16     # bf16 matmul operand dtype


def build_program(repeat=1):
    nc = bacc.Bacc("TRN2", target_bir_lowering=False)

    # ---------------- DRAM I/O ----------------
    q_in = nc.dram_tensor("queries", [BPC, D_DEC, 2, TQP], F8, kind="ExternalInput")
    k_in = nc.dram_tensor("keys", [BPC, D_ENC, TK + 6], DT_MM, kind="ExternalInput")
    kw1t_d = nc.dram_tensor("kw1t", [4, 128, 3, DH], DT_MM, kind="ExternalInput")
    kw2t_d = nc.dram_tensor("kw2t", [2, 128, 3, DH], DT_MM, kind="ExternalInput")
    qw1t_d = nc.dram_tensor("qw1t", [D_DEC, 7, DH], F8, kind="ExternalInput")
    qw2t_d = nc.dram_tensor("qw2t", [2, 128, 7, DH], F8, kind="ExternalInput")
    qw3t_d = nc.dram_tensor("qw3t", [2, 128, 7, DH], F8, kind="ExternalInput")
    kb1_d = nc.dram_tensor("kb1c", [2, 128, 1], F32, kind="ExternalInput")
    kb2_d = nc.dram_tensor("kb2c", [2, 128, 1], F32, kind="ExternalInput")
    qb1_d = nc.dram_tensor("qb1c", [2, 128, 1], F32, kind="ExternalInput")
    qb2_d = nc.dram_tensor("qb2c", [2, 128, 1], F32, kind="ExternalInput")
    qb3_d = nc.dram_tensor("qb3c", [2, 128, 1], F32, kind="ExternalInput")
    attn_out = nc.dram_tensor("attn_out", [BPC, TQ, TK2], F32, kind="ExternalOutput")
    logp_out = nc.dram_tensor("logp_out", [BPC, TQ, TK2], BF16, kind="ExternalOutput")

    with tile.TileContext(nc) as tc:
        for _ in range(repeat):
            _emit(nc, tc, q_in, k_in, kw1t_d, kw2t_d, qw1t_d, qw2t_d,
                  qw3t_d, kb1_d, kb2_d, qb1_d, qb2_d, qb3_d, attn_out, logp_out)
    nc.compile()
    return nc


def _emit(nc, tc, q_in, k_in, kw1t_d, kw2t_d, qw1t_d, qw2t_d, qw3t_d,
          kb1_d, kb2_d, qb1_d, qb2_d, qb3_d, attn_out, logp_out):
    from contextlib import ExitStack
    ctx = ExitStack()
    with ctx:
        singles = ctx.enter_context(tc.tile_pool(name="singles", bufs=1))
        p_in = ctx.enter_context(tc.tile_pool(name="p_in", bufs=2))
        p_mid = ctx.enter_context(tc.tile_pool(name="p_mid", bufs=2))
        p_soft = ctx.enter_context(tc.tile_pool(name="p_soft", bufs=3))
        p_small = ctx.enter_context(tc.tile_pool(name="p_small", bufs=8))
        pp_conv = ctx.enter_context(
            tc.tile_pool(name="pp_conv", bufs=2, space="PSUM"))
        pp_score = ctx.enter_context(
            tc.tile_pool(name="pp_score", bufs=2, space="PSUM"))

        # Pin the ACT LUT set that serves Prelu/Exp/Ln/Copy together.
        nc.scalar.add_instruction(mybir.InstLoadActFuncSet(
            name=nc.get_next_instruction_name(), ins=[], outs=[],
            act_func_set_id=ACT_SET_ALL))

        # ---------------- weights into SBUF (once) ----------------
        # Tiny bias tensors first (the first Prelu acts gate the psum-bank
        # recycle; 5KB must not queue behind 1.3MB of weights), then the
        # query-conv1 operands (473KB) so PE can start within a few us; the
        # larger key-side transfers stream in behind them.
        b_k1 = singles.tile([128, 2], F32)
        b_k2 = singles.tile([128, 2], F32)
        b_q1 = singles.tile([128, 2], F32)
        b_q2 = singles.tile([128, 2], F32)
        b_q3 = singles.tile([128, 2], F32)
        for sb_t, dr in ((b_q1, qb1_d), (b_k1, kb1_d), (b_k2, kb2_d),
                         (b_q2, qb2_d), (b_q3, qb3_d)):
            for h in range(2):
                nc.sync.dma_start(out=sb_t[:, h:h + 1], in_=dr[h])

        w_qw1 = singles.tile([128, 7, DH], F8)
        nc.sync.dma_start(out=w_qw1[:D_DEC], in_=qw1t_d[:])
        qpad0 = p_in.tile([128, 2, TQP], F8, tag="qpad")
        for r in range(4):
            nc.sync.dma_start(out=qpad0[20 * r:20 * (r + 1)],
                              in_=q_in[0, 20 * r:20 * (r + 1)])

        w_kw1 = singles.tile([128, 4, 3, DH], DT_MM)
        for c in range(4):
            for j in range(3):
                nc.sync.dma_start(out=w_kw1[:, c, j], in_=kw1t_d[c, :, j])
        kpad0 = p_in.tile([128, 4, TK + 6], DT_MM, tag="kpad")
        for c in range(4):
            for v in range(2):
                lo, hi = 259 * v, min(259 * (v + 1), TK + 6)
                nc.sync.dma_start(out=kpad0[:, c, lo:hi],
                                  in_=k_in[0, 128 * c:128 * (c + 1), lo:hi])

        w_qw2 = singles.tile([128, 2, 7, DH], F8)
        for c in range(2):
            nc.sync.dma_start(out=w_qw2[:, c], in_=qw2t_d[c])
        w_kw2 = singles.tile([128, 2, 3, DH], DT_MM)
        for c in range(2):
            nc.sync.dma_start(out=w_kw2[:, c], in_=kw2t_d[c])
        w_qw3 = singles.tile([128, 2, 7, DH], F8)
        for c in range(2):
            nc.sync.dma_start(out=w_qw3[:, c], in_=qw3t_d[c])

        ones_col = singles.tile([128, 1], BF16)   # lhsT for k2 reduction
        nc.vector.memset(ones_col, 1.0)

        # persistent padded intermediates; margins zeroed once
        k1pad = singles.tile([128, 2, TK1 + 6], DT_MM)
        q1pad = singles.tile([128, 2, TQP], F8)
        q2pad = singles.tile([128, 2, TQP], F8)
        for h in range(2):
            nc.vector.memset(k1pad[:, h, 0:3], 0.0)
            nc.vector.memset(k1pad[:, h, TK1 + 3:TK1 + 6], 0.0)
            nc.vector.memset(q1pad[:, h, 0:3], 0.0)
            nc.vector.memset(q1pad[:, h, TQ + 3:TQP], 0.0)
            nc.vector.memset(q2pad[:, h, 0:3], 0.0)
            nc.vector.memset(q2pad[:, h, TQ + 3:TQP], 0.0)

        nxt = (kpad0, qpad0)
        for b in range(BPC):
            nxt = _emit_batch(nc, tc, b,
                              q_in, k_in, attn_out, logp_out,
                              w_kw1, w_kw2, w_qw1, w_qw2, w_qw3,
                              b_k1, b_k2, b_q1, b_q2, b_q3,
                              ones_col, k1pad, q1pad, q2pad,
                              p_in, p_mid, p_soft, p_small, pp_conv, pp_score,
                              kpad_pre=nxt[0], qpad_pre=nxt[1])


def _emit_batch(nc, tc, b, q_in, k_in, attn_out, logp_out,
                w_kw1, w_kw2, w_qw1, w_qw2, w_qw3,
                b_k1, b_k2, b_q1, b_q2, b_q3, ones_col,
                k1pad, q1pad, q2pad,
                p_in, p_mid, p_soft, p_small, pp_conv, pp_score,
                kpad_pre=None, qpad_pre=None):
    mm = nc.tensor.matmul
    act = nc.scalar.activation

    # ---------------- queries path (fp8) ----------------
    # q_in row 0 = padded queries (fp8), row 1 = same shifted left by one:
    # a DoubleRow matmul over the two rows covers taps (j, j+1) at once.
    qpad = qpad_pre

    # qconv1 (fp8: 3 tap-pair DoubleRow matmuls + plain tap 6) -> q1pad fp8
    for g in range(2):  # pairs of 512-wide t-chunks, i-interleaved banks
        for h in range(2):
            ps = pp_conv.tile([128, 2, 512], F32, tag="conv")
            for p in range(3):
                for i in range(2):
                    t4 = 2 * g + i
                    mm(ps[:, i, :],
                       w_qw1[:D_DEC, 2 * p:2 * p + 2, 128 * h:128 * (h + 1)],
                       qpad[:D_DEC, :, 512 * t4 + 2 * p:512 * t4 + 2 * p + 512],
                       start=(p == 0), stop=False, perf_mode=DR)
            for i in range(2):
                t4 = 2 * g + i
                mm(ps[:, i, :],
                   w_qw1[:D_DEC, 6, 128 * h:128 * (h + 1)],
                   qpad[:D_DEC, 0, 512 * t4 + 6:512 * t4 + 6 + 512],
                   start=False, stop=True)
            act(q1pad[:, h, 3 + 1024 * g:3 + 1024 * (g + 1)], ps[:, :, :],
                AF.Prelu, bias=b_q1[:, h:h + 1], scale=1.0 / SW, alpha=SLOPE)

    # qconv2 (fp8 DoubleRow: both c chunks per matmul) -> q2pad fp8
    for g in range(2):
        for h in range(2):
            ps = pp_conv.tile([128, 2, 512], F32, tag="conv")
            for j in range(7):
                for i in range(2):
                    t4 = 2 * g + i
                    mm(ps[:, i, :],
                       w_qw2[:, :, j, 128 * h:128 * (h + 1)],
                       q1pad[:, :, 512 * t4 + j:512 * t4 + j + 512],
                       start=(j == 0), stop=(j == 6), perf_mode=DR)
            act(q2pad[:, h, 3 + 1024 * g:3 + 1024 * (g + 1)], ps[:, :, :],
                AF.Prelu, bias=b_q2[:, h:h + 1], scale=1.0 / SW, alpha=SLOPE)

    # qconv3 (fp8 DoubleRow) -> q3 bf16 (true units)
    q3 = p_mid.tile([128, 2, TQ], DT_MM, tag="q3")
    for g in range(2):
        for h in range(2):
            ps = pp_conv.tile([128, 2, 512], F32, tag="conv")
            for j in range(7):
                for i in range(2):
                    t4 = 2 * g + i
                    mm(ps[:, i, :],
                       w_qw3[:, :, j, 128 * h:128 * (h + 1)],
                       q2pad[:, :, 512 * t4 + j:512 * t4 + j + 512],
                       start=(j == 0), stop=(j == 6), perf_mode=DR)
            act(q3[:, h, 1024 * g:1024 * (g + 1)], ps[:, :, :],
                AF.Prelu, bias=b_q3[:, h:h + 1], scale=1.0 / SW, alpha=SLOPE)

    # ---------------- keys path (bf16) ----------------
    kpad = kpad_pre

    # key conv1: Cin=512, K=3, out [256, 516] -> k1pad with 3-margins
    for h in range(2):
        ps = pp_conv.tile([128, 2, 512], F32, tag="conv")
        for c in range(4):
            for j in range(3):
                for th in range(2):
                    mm(ps[:, th, :HT1],
                       w_kw1[:, c, j, 128 * h:128 * (h + 1)],
                       kpad[:, c, HT1 * th + j:HT1 * th + j + HT1],
                       start=(c == 0 and j == 0), stop=(c == 3 and j == 2))
        act(k1pad[:, h, 3:3 + TK1], ps[:, :, :HT1],
            AF.Prelu, bias=b_k1[:, h:h + 1], scale=1.0, alpha=SLOPE)

    # key conv2: Cin=256, K=3, out [256, 520]
    ksb = p_mid.tile([128, 2, TK2], DT_MM, tag="ksb")
    for h in range(2):
        ps = pp_conv.tile([128, 2, 512], F32, tag="conv")
        for c in range(2):
            for j in range(3):
                for sh in range(2):
                    mm(ps[:, sh, :HT2],
                       w_kw2[:, c, j, 128 * h:128 * (h + 1)],
                       k1pad[:, c, HT2 * sh + j:HT2 * sh + j + HT2],
                       start=(c == 0 and j == 0), stop=(c == 1 and j == 2))
        act(ksb[:, h, :], ps[:, :, :HT2],
            AF.Prelu, bias=b_k2[:, h:h + 1], scale=1.0, alpha=SLOPE)

    # k2[s] = sum_c k[c,s]^2 -> k2bc[p,s] = -0.5*k2 broadcast on all partitions
    ksq = p_mid.tile([128, 2, TK2], BF16, tag="ksq")
    nc.vector.tensor_mul(ksq[:, :, :], ksb[:, :, :], ksb[:, :, :])
    k2row = p_mid.tile([1, TK2], F32, tag="k2row")
    ps2 = pp_score.tile([1, 2, 512], F32, tag="score")
    for sh in range(2):
        for c in range(2):
            mm(ps2[:, sh, :HT2], ones_col[:, :],
               ksq[:, c, HT2 * sh:HT2 * sh + HT2],
               start=(c == 0), stop=(c == 1))
    act(k2row[:, :].rearrange("p (a b) -> p a b", a=2), ps2[:, :, :HT2],
        AF.Copy, bias=0.0, scale=-0.5)
    k2bc = p_mid.tile([128, TK2], F32, tag="k2bc")
    nc.gpsimd.partition_broadcast(k2bc[:, :], k2row[:, :])

    # prefetch next batch's inputs BEFORE the score-phase output burst so
    # the input DMAs aren't queued behind ~4MB of attn/logp writes
    kpad_n = qpad_n = None
    if b + 1 < BPC:
        ni = b + 1 if k_in.shape[0] > 1 else 0
        qpad_n = p_in.tile([128, 2, TQP], F8, tag="qpad")
        for r in range(4):
            nc.sync.dma_start(out=qpad_n[20 * r:20 * (r + 1)],
                              in_=q_in[ni, 20 * r:20 * (r + 1)])
        kpad_n = p_in.tile([128, 4, TK + 6], DT_MM, tag="kpad")
        for c in range(4):
            for v in range(2):
                lo, hi = 259 * v, min(259 * (v + 1), TK + 6)
                nc.sync.dma_start(out=kpad_n[:, c, lo:hi],
                                  in_=k_in[ni, 128 * c:128 * (c + 1), lo:hi])

    # ---------------- scores + softmax ----------------
    k2bc2 = k2bc[:, :].rearrange("p (a b) -> p a b", a=2)
    for t in range(TQ // 128):
        sp = pp_score.tile([128, 2, 512], F32, tag="score", name=f"sp{b}_{t}")
        for c in range(2):
            for sh in range(2):
                mm(sp[:, sh, :HT2],
                   q3[:, c, 128 * t:128 * (t + 1)],
                   ksb[:, c, HT2 * sh:HT2 * sh + HT2],
                   start=(c == 0), stop=(c == 1))
        # raw = qk - 0.5*k2 (DVE broadcast add), drained to SBUF so the DVE
        # add is the ONLY psum reader and the score bank frees after one hop.
        raw_sb = p_soft.tile([128, TK2], F32, tag="raw")
        nc.vector.tensor_add(raw_sb[:, :].rearrange("p (a b) -> p a b", a=2),
                             sp[:, :, :HT2], k2bc2)

        esb = p_soft.tile([128, TK2], F32, tag="esb")
        z = p_small.tile([128, 1], F32, tag="z")
        act(esb, raw_sb, AF.Exp, bias=0.0, scale=SC, accum_out=z)
        rz = p_small.tile([128, 1], F32, tag="rz")
        nc.vector.reciprocal(rz, z)
        attn_sb = p_soft.tile([128, TK2], F32, tag="attn")
        nc.vector.tensor_scalar_mul(attn_sb, esb, rz)
        # logp = SC*raw + ln(1/z)  (ln(rz) = -ln z; tiny scalar act)
        lnrz = p_small.tile([128, 1], F32, tag="lnrz")
        act(lnrz, rz, AF.Ln)
        logp_sb = p_soft.tile([128, TK2], BF16, tag="logp")
        nc.vector.tensor_scalar(
            out=logp_sb, in0=raw_sb, scalar1=SC, scalar2=lnrz,
            op0=mybir.AluOpType.mult, op1=mybir.AluOpType.add)

        for r in range(4):
            nc.sync.dma_start(
                out=attn_out[b, 128 * t + 32 * r:128 * t + 32 * (r + 1), :],
                in_=attn_sb[32 * r:32 * (r + 1)])
        for r in range(2):
            nc.sync.dma_start(
                out=logp_out[b, 128 * t + 64 * r:128 * t + 64 * (r + 1), :],
                in_=logp_sb[64 * r:64 * (r + 1)])

    return (kpad_n, qpad_n)


def build_timing_program(repeat=1):
    """Same compute, but single-batch external inputs reused for all batches,
    outputs to Internal DRAM scratch + tiny canary output: removes the
    hundreds-of-MB per-call transfer so wall-clock deltas measure exec."""
    nc = bacc.Bacc("TRN2", target_bir_lowering=False)
    q_in = nc.dram_tensor("queries", [1, D_DEC, 2, TQP], F8, kind="ExternalInput")
    k_in = nc.dram_tensor("keys", [1, D_ENC, TK + 6], DT_MM, kind="ExternalInput")
    kw1t_d = nc.dram_tensor("kw1t", [4, 128, 3, DH], DT_MM, kind="ExternalInput")
    kw2t_d = nc.dram_tensor("kw2t", [2, 128, 3, DH], DT_MM, kind="ExternalInput")
    qw1t_d = nc.dram_tensor("qw1t", [D_DEC, 7, DH], F8, kind="ExternalInput")
    qw2t_d = nc.dram_tensor("qw2t", [2, 128, 7, DH], F8, kind="ExternalInput")
    qw3t_d = nc.dram_tensor("qw3t", [2, 128, 7, DH], F8, kind="ExternalInput")
    kb1_d = nc.dram_tensor("kb1c", [2, 128, 1], F32, kind="ExternalInput")
    kb2_d = nc.dram_tensor("kb2c", [2, 128, 1], F32, kind="ExternalInput")
    qb1_d = nc.dram_tensor("qb1c", [2, 128, 1], F32, kind="ExternalInput")
    qb2_d = nc.dram_tensor("qb2c", [2, 128, 1], F32, kind="ExternalInput")
    qb3_d = nc.dram_tensor("qb3c", [2, 128, 1], F32, kind="ExternalInput")
    attn_s = nc.dram_tensor("attn_s", [BPC, TQ, TK2], F32)
    logp_s = nc.dram_tensor("logp_s", [BPC, TQ, TK2], BF16)
    canary = nc.dram_tensor("canary", [1, 16], F32, kind="ExternalOutput")

    with tile.TileContext(nc) as tc:
        for _ in range(repeat):
            _emit(nc, tc, q_in, k_in, kw1t_d, kw2t_d, qw1t_d, qw2t_d,
                  qw3t_d, kb1_d, kb2_d, qb1_d, qb2_d, qb3_d, attn_s, logp_s)
        with tc.tile_pool(name="canary_p", bufs=1) as cp:
            ct = cp.tile([1, 16], F32)
            nc.sync.dma_start(out=ct[:, :], in_=attn_s[0, 0:1, 0:16])
            nc.sync.dma_start(out=canary[:, :], in_=ct[:, :])
    nc.compile()
    return nc


def timing_in_maps(in_maps):
    out = []
    for m in in_maps:
        m2 = dict(m)
        m2["queries"] = m["queries"][0:1]
        m2["keys"] = m["keys"][0:1]
        out.append(m2)
    return out


_PROGRAM = None


def _get_program():
    global _PROGRAM
    if _PROGRAM is None:
        _PROGRAM = build_program()
    return _PROGRAM


def prep_inputs(queries, keys, kw1, kb1, kw2, kb2, qw1, qb1, qw2, qb2, qw3, qb3):
    """Build the 8 per-core input maps from full-size inputs."""
    f = np.float32
    fm = mybir.dt.np(DT_MM)
    f8 = mybir.dt.np(F8)
    kw1t = np.ascontiguousarray(np.transpose(kw1, (1, 2, 0)).reshape(4, 128, 3, DH), fm)
    kw2t = np.ascontiguousarray(np.transpose(kw2, (1, 2, 0)).reshape(2, 128, 3, DH), fm)
    qw1t = np.clip(np.transpose(qw1, (1, 2, 0)) * SW, -240, 240).astype(f8)
    qw2t = np.clip(np.transpose(qw2, (1, 2, 0)).reshape(2, 128, 7, DH) * SW,
                   -240, 240).astype(f8)
    qw3t = np.clip(np.transpose(qw3, (1, 2, 0)).reshape(2, 128, 7, DH) * SW,
                   -240, 240).astype(f8)
    shared = dict(
        kw1t=kw1t, kw2t=kw2t, qw1t=np.ascontiguousarray(qw1t),
        qw2t=np.ascontiguousarray(qw2t), qw3t=np.ascontiguousarray(qw3t),
        kb1c=np.ascontiguousarray(kb1.reshape(2, 128, 1), f),
        kb2c=np.ascontiguousarray(kb2.reshape(2, 128, 1), f),
        qb1c=np.ascontiguousarray(qb1.reshape(2, 128, 1), f),
        qb2c=np.ascontiguousarray(qb2.reshape(2, 128, 1), f),
        qb3c=np.ascontiguousarray(qb3.reshape(2, 128, 1), f),
    )
    B = queries.shape[0]
    q8v = np.clip(queries, -240, 240).astype(f8)
    qp = np.zeros((B, D_DEC, 2, TQP), f8)
    qp[:, :, 0, 3:TQ + 3] = q8v
    qp[:, :, 1, 2:TQ + 2] = q8v
    kp = np.zeros((B, D_ENC, TK + 6), fm)
    kp[:, :, 3:TK + 3] = keys
    in_maps = []
    for i in range(N_CORES):
        m = dict(shared)
        m["queries"] = np.ascontiguousarray(qp[BPC * i:BPC * (i + 1)])
        m["keys"] = np.ascontiguousarray(kp[BPC * i:BPC * (i + 1)])
        in_maps.append(m)
    return in_maps


def run(in_maps, **kwargs):
    nc = _get_program()
    return run_bass_kernel_spmd(nc, in_maps, core_ids=list(range(N_CORES)), **kwargs)


def kernel(queries, keys, kw1, kb1, kw2, kb2, qw1, qb1, qw2, qb2, qw3, qb3,
           **kwargs):
    in_maps = prep_inputs(queries, keys, kw1, kb1, kw2, kb2,
                          qw1, qb1, qw2, qb2, qw3, qb3)
    res = run(in_maps)
    attn = np.concatenate([np.asarray(r["attn_out"], np.float32)
                           for r in res.results], axis=0)
    logp = np.concatenate([np.asarray(r["logp_out"], np.float32)
                           for r in res.results], axis=0)
    B = attn.shape[0]
    return attn.reshape(B, 1, TQ, TK2), logp.reshape(B, 1, TQ, TK2)


# revision 31
# speedup vs baseline: 1.5606x; 1.5606x over previous
"""Trainium2 Bass kernel for the Aligner2 problem.

Computes, for each batch b:
  k = LReLU(conv3(LReLU(conv3(keys))))        # [256, 520] (pad 3, kernel 3 twice)
  q = LReLU(conv7(LReLU(conv7(LReLU(conv7(queries))))))  # [256, 2048]
  raw[t,s]  = sum_c q[c,t] k[c,s] - 0.5*k2[s]
  l = 2*TEMP*raw   (the -TEMP*q2 term cancels in log_softmax)
  logp = l - logsumexp_s(l);  attn = exp(logp)

v2: query convs 2+3 run in fp8e4 with DoubleRow perf mode (2 k-tiles per
matmul, 0.5 cyc/row); the -0.5*k2 row is broadcast-added on DVE instead of
per-tile PE matmuls; logp computed as SC*raw - ln(z) on DVE (no big Ln on
scalar). Everything else (key path, qconv1, qk scores) stays bf16.

Sharded data-parallel over batch across 8 NeuronCores (4 batches/core).
"""
import numpy as np

import concourse.bass as bass
import concourse.bacc as bacc
import concourse.tile as tile
from concourse import mybir
from concourse.bass_utils import run_bass_kernel_spmd

F32 = mybir.dt.float32
BF16 = mybir.dt.bfloat16
F8 = mybir.dt.float8e4
AF = mybir.ActivationFunctionType
DR = mybir.MatmulPerfMode.DoubleRow

SLOPE = 0.3
TEMPERATURE = 0.0005
SC = 2.0 * TEMPERATURE  # scale applied to the raw PE scores
SW = 256.0              # fp8 weight pre-scale (host side); act descales

BPC = 4          # batches per core
N_CORES = 8
D_DEC, TQ = 80, 2048
D_ENC, TK = 512, 512
DH = 256
TK1 = TK + 4     # 516 after key conv1 (kernel 3, pad 3)
TK2 = TK + 8     # 520 after key conv2
HT1 = TK1 // 2   # 258
HT2 = TK2 // 2   # 260
TQP = TQ + 16    # fp8 padded q activations: stride multiple of 16

# act_info.json set containing Prelu, Exp, Ln, Copy, Identity together
ACT_SET_ALL = 6  # natural_log_exp_and_others

DT_MM = BF16     # bf16 matmul operand dtype


def build_program(repeat=1):
    nc = bacc.Bacc("TRN2", target_bir_lowering=False)

    # ---------------- DRAM I/O ----------------
    q_in = nc.dram_tensor("queries", [BPC, D_DEC, 2, TQP], F8, kind="ExternalInput")
    k_in = nc.dram_tensor("keys", [BPC, D_ENC, TK + 6], DT_MM, kind="ExternalInput")
    kw1t_d = nc.dram_tensor("kw1t", [4, 128, 3, DH], DT_MM, kind="ExternalInput")
    kw2t_d = nc.dram_tensor("kw2t", [2, 128, 3, DH], DT_MM, kind="ExternalInput")
    qw1t_d = nc.dram_tensor("qw1t", [D_DEC, 7, DH], F8, kind="ExternalInput")
    qw2t_d = nc.dram_tensor("qw2t", [2, 128, 7, DH], F8, kind="ExternalInput")
    qw3t_d = nc.dram_tensor("qw3t", [2, 128, 7, DH], F8, kind="ExternalInput")
    kb1_d = nc.dram_tensor("kb1c", [2, 128, 1], F32, kind="ExternalInput")
    kb2_d = nc.dram_tensor("kb2c", [2, 128, 1], F32, kind="ExternalInput")
    qb1_d = nc.dram_tensor("qb1c", [2, 128, 1], F32, kind="ExternalInput")
    qb2_d = nc.dram_tensor("qb2c", [2, 128, 1], F32, kind="ExternalInput")
    qb3_d = nc.dram_tensor("qb3c", [2, 128, 1], F32, kind="ExternalInput")
    attn_out = nc.dram_tensor("attn_out", [BPC, TQ, TK2], F32, kind="ExternalOutput")
    logp_out = nc.dram_tensor("logp_out", [BPC, TQ, TK2], BF16, kind="ExternalOutput")

    with tile.TileContext(nc) as tc:
        for _ in range(repeat):
            _emit(nc, tc, q_in, k_in, kw1t_d, kw2t_d, qw1t_d, qw2t_d,
                  qw3t_d, kb1_d, kb2_d, qb1_d, qb2_d, qb3_d, attn_out, logp_out)
    nc.compile()
    return nc


def _emit(nc, tc, q_in, k_in, kw1t_d, kw2t_d, qw1t_d, qw2t_d, qw3t_d,
          kb1_d, kb2_d, qb1_d, qb2_d, qb3_d, attn_out, logp_out):
    from contextlib import ExitStack
    ctx = ExitStack()
    with ctx:
        singles = ctx.enter_context(tc.tile_pool(name="singles", bufs=1))
        p_in = ctx.enter_context(tc.tile_pool(name="p_in", bufs=2))
        p_mid = ctx.enter_context(tc.tile_pool(name="p_mid", bufs=2))
        p_soft = ctx.enter_context(tc.tile_pool(name="p_soft", bufs=3))
        p_small = ctx.enter_context(tc.tile_pool(name="p_small", bufs=8))
        pp_conv = ctx.enter_context(
            tc.tile_pool(name="pp_conv", bufs=2, space="PSUM"))
        pp_score = ctx.enter_context(
            tc.tile_pool(name="pp_score", bufs=2, space="PSUM"))

        # Pin the ACT LUT set that serves Prelu/Exp/Ln/Copy together.
        nc.scalar.add_instruction(mybir.InstLoadActFuncSet(
            name=nc.get_next_instruction_name(), ins=[], outs=[],
            act_func_set_id=ACT_SET_ALL))

        # ---------------- weights into SBUF (once) ----------------
        # Tiny bias tensors first (the first Prelu acts gate the psum-bank
        # recycle; 5KB must not queue behind 1.3MB of weights), then the
        # query-conv1 operands (473KB) so PE can start within a few us; the
        # larger key-side transfers stream in behind them.
        b_k1 = singles.tile([128, 2], F32)
        b_k2 = singles.tile([128, 2], F32)
        b_q1 = singles.tile([128, 2], F32)
        b_q2 = singles.tile([128, 2], F32)
        b_q3 = singles.tile([128, 2], F32)
        for sb_t, dr in ((b_q1, qb1_d), (b_k1, kb1_d), (b_k2, kb2_d),
                         (b_q2, qb2_d), (b_q3, qb3_d)):
            for h in range(2):
                nc.sync.dma_start(out=sb_t[:, h:h + 1], in_=dr[h])

        w_qw1 = singles.tile([128, 7, DH], F8)
        nc.sync.dma_start(out=w_qw1[:D_DEC], in_=qw1t_d[:])
        qpad0 = p_in.tile([128, 2, TQP], F8, tag="qpad")
        nc.sync.dma_start(out=qpad0[:D_DEC], in_=q_in[0])

        w_kw1 = singles.tile([128, 4, 3, DH], DT_MM)
        for c in range(4):
            for j in range(3):
                nc.sync.dma_start(out=w_kw1[:, c, j], in_=kw1t_d[c, :, j])
        kpad0 = p_in.tile([128, 4, TK + 6], DT_MM, tag="kpad")
        for c in range(4):
            nc.sync.dma_start(out=kpad0[:, c, :],
                              in_=k_in[0, 128 * c:128 * (c + 1), :])

        w_qw2 = singles.tile([128, 2, 7, DH], F8)
        for c in range(2):
            nc.sync.dma_start(out=w_qw2[:, c], in_=qw2t_d[c])
        w_kw2 = singles.tile([128, 2, 3, DH], DT_MM)
        for c in range(2):
            nc.sync.dma_start(out=w_kw2[:, c], in_=kw2t_d[c])
        w_qw3 = singles.tile([128, 2, 7, DH], F8)
        for c in range(2):
            nc.sync.dma_start(out=w_qw3[:, c], in_=qw3t_d[c])

        ones_col = singles.tile([128, 1], BF16)   # lhsT for k2 reduction
        nc.vector.memset(ones_col, 1.0)

        # persistent padded intermediates; margins zeroed once
        k1pad = singles.tile([128, 2, TK1 + 6], DT_MM)
        q1pad = singles.tile([128, 2, TQP], F8)
        q2pad = singles.tile([128, 2, TQP], F8)
        for h in range(2):
            nc.vector.memset(k1pad[:, h, 0:3], 0.0)
            nc.vector.memset(k1pad[:, h, TK1 + 3:TK1 + 6], 0.0)
            nc.vector.memset(q1pad[:, h, 0:3], 0.0)
            nc.vector.memset(q1pad[:, h, TQ + 3:TQP], 0.0)
            nc.vector.memset(q2pad[:, h, 0:3], 0.0)
            nc.vector.memset(q2pad[:, h, TQ + 3:TQP], 0.0)

        nxt = (kpad0, qpad0)
        for b in range(BPC):
            nxt = _emit_batch(nc, tc, b,
                              q_in, k_in, attn_out, logp_out,
                              w_kw1, w_kw2, w_qw1, w_qw2, w_qw3,
                              b_k1, b_k2, b_q1, b_q2, b_q3,
                              ones_col, k1pad, q1pad, q2pad,
                              p_in, p_mid, p_soft, p_small, pp_conv, pp_score,
                              kpad_pre=nxt[0], qpad_pre=nxt[1])


def _emit_batch(nc, tc, b, q_in, k_in, attn_out, logp_out,
                w_kw1, w_kw2, w_qw1, w_qw2, w_qw3,
                b_k1, b_k2, b_q1, b_q2, b_q3, ones_col,
                k1pad, q1pad, q2pad,
                p_in, p_mid, p_soft, p_small, pp_conv, pp_score,
                kpad_pre=None, qpad_pre=None):
    mm = nc.tensor.matmul
    act = nc.scalar.activation

    # ---------------- queries path (fp8) ----------------
    # q_in row 0 = padded queries (fp8), row 1 = same shifted left by one:
    # a DoubleRow matmul over the two rows covers taps (j, j+1) at once.
    qpad = qpad_pre

    # qconv1 (fp8: 3 tap-pair DoubleRow matmuls + plain tap 6) -> q1pad fp8
    for g in range(2):  # pairs of 512-wide t-chunks, i-interleaved banks
        for h in range(2):
            ps = pp_conv.tile([128, 2, 512], F32, tag="conv")
            for p in range(3):
                for i in range(2):
                    t4 = 2 * g + i
                    mm(ps[:, i, :],
                       w_qw1[:D_DEC, 2 * p:2 * p + 2, 128 * h:128 * (h + 1)],
                       qpad[:D_DEC, :, 512 * t4 + 2 * p:512 * t4 + 2 * p + 512],
                       start=(p == 0), stop=False, perf_mode=DR)
            for i in range(2):
                t4 = 2 * g + i
                mm(ps[:, i, :],
                   w_qw1[:D_DEC, 6, 128 * h:128 * (h + 1)],
                   qpad[:D_DEC, 0, 512 * t4 + 6:512 * t4 + 6 + 512],
                   start=False, stop=True)
            act(q1pad[:, h, 3 + 1024 * g:3 + 1024 * (g + 1)], ps[:, :, :],
                AF.Prelu, bias=b_q1[:, h:h + 1], scale=1.0 / SW, alpha=SLOPE)

    # qconv2 (fp8 DoubleRow: both c chunks per matmul) -> q2pad fp8
    for g in range(2):
        for h in range(2):
            ps = pp_conv.tile([128, 2, 512], F32, tag="conv")
            for j in range(7):
                for i in range(2):
                    t4 = 2 * g + i
                    mm(ps[:, i, :],
                       w_qw2[:, :, j, 128 * h:128 * (h + 1)],
                       q1pad[:, :, 512 * t4 + j:512 * t4 + j + 512],
                       start=(j == 0), stop=(j == 6), perf_mode=DR)
            act(q2pad[:, h, 3 + 1024 * g:3 + 1024 * (g + 1)], ps[:, :, :],
                AF.Prelu, bias=b_q2[:, h:h + 1], scale=1.0 / SW, alpha=SLOPE)

    # qconv3 (fp8 DoubleRow) -> q3 bf16 (true units)
    q3 = p_mid.tile([128, 2, TQ], DT_MM, tag="q3")
    for g in range(2):
        for h in range(2):
            ps = pp_conv.tile([128, 2, 512], F32, tag="conv")
            for j in range(7):
                for i in range(2):
                    t4 = 2 * g + i
                    mm(ps[:, i, :],
                       w_qw3[:, :, j, 128 * h:128 * (h + 1)],
                       q2pad[:, :, 512 * t4 + j:512 * t4 + j + 512],
                       start=(j == 0), stop=(j == 6), perf_mode=DR)
            act(q3[:, h, 1024 * g:1024 * (g + 1)], ps[:, :, :],
                AF.Prelu, bias=b_q3[:, h:h + 1], scale=1.0 / SW, alpha=SLOPE)

    # ---------------- keys path (bf16) ----------------
    kpad = kpad_pre

    # key conv1: Cin=512, K=3, out [256, 516] -> k1pad with 3-margins
    for h in range(2):
        ps = pp_conv.tile([128, 2, 512], F32, tag="conv")
        for c in range(4):
            for j in range(3):
                for th in range(2):
                    mm(ps[:, th, :HT1],
                       w_kw1[:, c, j, 128 * h:128 * (h + 1)],
                       kpad[:, c, HT1 * th + j:HT1 * th + j + HT1],
                       start=(c == 0 and j == 0), stop=(c == 3 and j == 2))
        act(k1pad[:, h, 3:3 + TK1], ps[:, :, :HT1],
            AF.Prelu, bias=b_k1[:, h:h + 1], scale=1.0, alpha=SLOPE)

    # key conv2: Cin=256, K=3, out [256, 520]
    ksb = p_mid.tile([128, 2, TK2], DT_MM, tag="ksb")
    for h in range(2):
        ps = pp_conv.tile([128, 2, 512], F32, tag="conv")
        for c in range(2):
            for j in range(3):
                for sh in range(2):
                    mm(ps[:, sh, :HT2],
                       w_kw2[:, c, j, 128 * h:128 * (h + 1)],
                       k1pad[:, c, HT2 * sh + j:HT2 * sh + j + HT2],
                       start=(c == 0 and j == 0), stop=(c == 1 and j == 2))
        act(ksb[:, h, :], ps[:, :, :HT2],
            AF.Prelu, bias=b_k2[:, h:h + 1], scale=1.0, alpha=SLOPE)

    # k2[s] = sum_c k[c,s]^2 -> k2bc[p,s] = -0.5*k2 broadcast on all partitions
    ksq = p_mid.tile([128, 2, TK2], BF16, tag="ksq")
    nc.vector.tensor_mul(ksq[:, :, :], ksb[:, :, :], ksb[:, :, :])
    k2row = p_mid.tile([1, TK2], F32, tag="k2row")
    ps2 = pp_score.tile([1, 2, 512], F32, tag="score")
    for sh in range(2):
        for c in range(2):
            mm(ps2[:, sh, :HT2], ones_col[:, :],
               ksq[:, c, HT2 * sh:HT2 * sh + HT2],
               start=(c == 0), stop=(c == 1))
    act(k2row[:, :].rearrange("p (a b) -> p a b", a=2), ps2[:, :, :HT2],
        AF.Copy, bias=0.0, scale=-0.5)
    k2bc = p_mid.tile([128, TK2], F32, tag="k2bc")
    nc.gpsimd.partition_broadcast(k2bc[:, :], k2row[:, :])

    # prefetch next batch's inputs BEFORE the score-phase output burst so
    # the input DMAs aren't queued behind ~4MB of attn/logp writes
    kpad_n = qpad_n = None
    if b + 1 < BPC:
        ni = b + 1 if k_in.shape[0] > 1 else 0
        qpad_n = p_in.tile([128, 2, TQP], F8, tag="qpad")
        nc.sync.dma_start(out=qpad_n[:D_DEC], in_=q_in[ni])
        kpad_n = p_in.tile([128, 4, TK + 6], DT_MM, tag="kpad")
        for c in range(4):
            nc.sync.dma_start(out=kpad_n[:, c, :],
                              in_=k_in[ni, 128 * c:128 * (c + 1), :])

    # ---------------- scores + softmax ----------------
    k2bc2 = k2bc[:, :].rearrange("p (a b) -> p a b", a=2)
    for t in range(TQ // 128):
        sp = pp_score.tile([128, 2, 512], F32, tag="score", name=f"sp{b}_{t}")
        for c in range(2):
            for sh in range(2):
                mm(sp[:, sh, :HT2],
                   q3[:, c, 128 * t:128 * (t + 1)],
                   ksb[:, c, HT2 * sh:HT2 * sh + HT2],
                   start=(c == 0), stop=(c == 1))
        # raw = qk - 0.5*k2 (DVE broadcast add), drained to SBUF so the DVE
        # add is the ONLY psum reader and the score bank frees after one hop.
        raw_sb = p_soft.tile([128, TK2], F32, tag="raw")
        nc.vector.tensor_add(raw_sb[:, :].rearrange("p (a b) -> p a b", a=2),
                             sp[:, :, :HT2], k2bc2)

        esb = p_soft.tile([128, TK2], F32, tag="esb")
        z = p_small.tile([128, 1], F32, tag="z")
        act(esb, raw_sb, AF.Exp, bias=0.0, scale=SC, accum_out=z)
        rz = p_small.tile([128, 1], F32, tag="rz")
        nc.vector.reciprocal(rz, z)
        attn_sb = p_soft.tile([128, TK2], F32, tag="attn")
        nc.vector.tensor_scalar_mul(attn_sb, esb, rz)
        # logp = SC*raw + ln(1/z)  (ln(rz) = -ln z; tiny scalar act)
        lnrz = p_small.tile([128, 1], F32, tag="lnrz")
        act(lnrz, rz, AF.Ln)
        logp_sb = p_soft.tile([128, TK2], BF16, tag="logp")
        nc.vector.tensor_scalar(
            out=logp_sb, in0=raw_sb, scalar1=SC, scalar2=lnrz,
            op0=mybir.AluOpType.mult, op1=mybir.AluOpType.add)

        nc.sync.dma_start(out=attn_out[b, 128 * t:128 * (t + 1), :], in_=attn_sb)
        nc.sync.dma_start(out=logp_out[b, 128 * t:128 * (t + 1), :], in_=logp_sb)

    return (kpad_n, qpad_n)


def build_timing_program(repeat=1):
    """Same compute, but single-batch external inputs reused for all batches,
    outputs to Internal DRAM scratch + tiny canary output: removes the
    hundreds-of-MB per-call transfer so wall-clock deltas measure exec."""
    nc = bacc.Bacc("TRN2", target_bir_lowering=False)
    q_in = nc.dram_tensor("queries", [1, D_DEC, 2, TQP], F8, kind="ExternalInput")
    k_in = nc.dram_tensor("keys", [1, D_ENC, TK + 6], DT_MM, kind="ExternalInput")
    kw1t_d = nc.dram_tensor("kw1t", [4, 128, 3, DH], DT_MM, kind="ExternalInput")
    kw2t_d = nc.dram_tensor("kw2t", [2, 128, 3, DH], DT_MM, kind="ExternalInput")
    qw1t_d = nc.dram_tensor("qw1t", [D_DEC, 7, DH], F8, kind="ExternalInput")
    qw2t_d = nc.dram_tensor("qw2t", [2, 128, 7, DH], F8, kind="ExternalInput")
    qw3t_d = nc.dram_tensor("qw3t", [2, 128, 7, DH], F8, kind="ExternalInput")
    kb1_d = nc.dram_tensor("kb1c", [2, 128, 1], F32, kind="ExternalInput")
    kb2_d = nc.dram_tensor("kb2c", [2, 128, 1], F32, kind="ExternalInput")
    qb1_d = nc.dram_tensor("qb1c", [2, 128, 1], F32, kind="ExternalInput")
    qb2_d = nc.dram_tensor("qb2c", [2, 128, 1], F32, kind="ExternalInput")
    qb3_d = nc.dram_tensor("qb3c", [2, 128, 1], F32, kind="ExternalInput")
    attn_s = nc.dram_tensor("attn_s", [BPC, TQ, TK2], F32)
    logp_s = nc.dram_tensor("logp_s", [BPC, TQ, TK2], BF16)
    canary = nc.dram_tensor("canary", [1, 16], F32, kind="ExternalOutput")

    with tile.TileContext(nc) as tc:
        for _ in range(repeat):
            _emit(nc, tc, q_in, k_in, kw1t_d, kw2t_d, qw1t_d, qw2t_d,
                  qw3t_d, kb1_d, kb2_d, qb1_d, qb2_d, qb3_d, attn_s, logp_s)
        with tc.tile_pool(name="canary_p", bufs=1) as cp:
            ct = cp.tile([1, 16], F32)
            nc.sync.dma_start(out=ct[:, :], in_=attn_s[0, 0:1, 0:16])
            nc.sync.dma_start(out=canary[:, :], in_=ct[:, :])
    nc.compile()
    return nc


def timing_in_maps(in_maps):
    out = []
    for m in in_maps:
        m2 = dict(m)
        m2["queries"] = m["queries"][0:1]
        m2["keys"] = m["keys"][0:1]
        out.append(m2)
    return out


_PROGRAM = None


def _get_program():
    global _PROGRAM
    if _PROGRAM is None:
        _PROGRAM = build_program()
    return _PROGRAM


def prep_inputs(queries, keys, kw1, kb1, kw2, kb2, qw1, qb1, qw2, qb2, qw3, qb3):
    """Build the 8 per-core input maps from full-size inputs."""
    f = np.float32
    fm = mybir.dt.np(DT_MM)
    f8 = mybir.dt.np(F8)
    kw1t = np.ascontiguousarray(np.transpose(kw1, (1, 2, 0)).reshape(4, 128, 3, DH), fm)
    kw2t = np.ascontiguousarray(np.transpose(kw2, (1, 2, 0)).reshape(2, 128, 3, DH), fm)
    qw1t = np.clip(np.transpose(qw1, (1, 2, 0)) * SW, -240, 240).astype(f8)
    qw2t = np.clip(np.transpose(qw2, (1, 2, 0)).reshape(2, 128, 7, DH) * SW,
                   -240, 240).astype(f8)
    qw3t = np.clip(np.transpose(qw3, (1, 2, 0)).reshape(2, 128, 7, DH) * SW,
                   -240, 240).astype(f8)
    shared = dict(
        kw1t=kw1t, kw2t=kw2t, qw1t=np.ascontiguousarray(qw1t),
        qw2t=np.ascontiguousarray(qw2t), qw3t=np.ascontiguousarray(qw3t),
        kb1c=np.ascontiguousarray(kb1.reshape(2, 128, 1), f),
        kb2c=np.ascontiguousarray(kb2.reshape(2, 128, 1), f),
        qb1c=np.ascontiguousarray(qb1.reshape(2, 128, 1), f),
        qb2c=np.ascontiguousarray(qb2.reshape(2, 128, 1), f),
        qb3c=np.ascontiguousarray(qb3.reshape(2, 128, 1), f),
    )
    B = queries.shape[0]
    q8v = np.clip(queries, -240, 240).astype(f8)
    qp = np.zeros((B, D_DEC, 2, TQP), f8)
    qp[:, :, 0, 3:TQ + 3] = q8v
    qp[:, :, 1, 2:TQ + 2] = q8v
    kp = np.zeros((B, D_ENC, TK + 6), fm)
    kp[:, :, 3:TK + 3] = keys
    in_maps = []
    for i in range(N_CORES):
        m = dict(shared)
        m["queries"] = np.ascontiguousarray(qp[BPC * i:BPC * (i + 1)])
        m["keys"] = np.ascontiguousarray(kp[BPC * i:BPC * (i + 1)])
        in_maps.append(m)
    return in_maps


def run(in_maps, **kwargs):
    nc = _get_program()
    return run_bass_kernel_spmd(nc, in_maps, core_ids=list(range(N_CORES)), **kwargs)


def kernel(queries, keys, kw1, kb1, kw2, kb2, qw1, qb1, qw2, qb2, qw3, qb3,
           **kwargs):
    in_maps = prep_inputs(queries, keys, kw1, kb1, kw2, kb2,
                          qw1, qb1, qw2, qb2, qw3, qb3)
    res = run(in_maps)
    attn = np.concatenate([np.asarray(r["attn_out"], np.float32)
                           for r in res.results], axis=0)
    logp = np.concatenate([np.asarray(r["logp_out"], np.float32)
                           for r in res.results], axis=0)
    B = attn.shape[0]
    return attn.reshape(B, 1, TQ, TK2), logp.reshape(B, 1, TQ, TK2)
